# revision 10
# baseline (speedup 1.0000x reference)
"""Multi-head attention (B=8, N=1024, C=768, H=12, D=64) on 8 TRN2 NeuronCores.

Strategy: pure data-parallel over batch (B == n_cores == 8), no collectives.
Each core computes full 12-head attention for one batch element, operating in
a fully transposed layout (channels on SBUF partitions) so that no on-device
transposes are needed:

  per core:  xT=[C,N] -> QT,KT=[C,N], V=[N,C] (+ ones col)
             per (head, nq-block, nk-tile): S^T = K_h Q_h^T (PSUM, f32)
             S^T += 8*bias^T (DVE, bf16 bias);  P^T = exp(0.125*S^T) (ACT->bf16)
             PV:  [V_h | 1]^T @ P^T -> rows 0:64 = out_h^T (unnorm), row 64 = sum
             batched reciprocal of all softmax sums, broadcast multiply,
             out^T = Wp @ attnT + bp  -> DMA out, host transposes back.

Matmuls run in bf16 (f32 PSUM accumulation); softmax math in f32.
"""

import os
import sys
import numpy as np

for _p in ("/opt/trn_rl_repo", "/root/.axon_site/_ro/trn_rl_repo"):
    if os.path.isdir(_p) and _p not in sys.path:
        sys.path.append(_p)

import ml_dtypes

BF16 = ml_dtypes.bfloat16

B, N, C = 8, 1024, 768
H, D = 12, 64
CT = C // 128        # 6 channel tiles
NT = N // 128        # 8 key tiles
NQB = N // 512       # 2 query blocks
F = 512

_cache = {}


def _build():
    import concourse.bass as bass
    import concourse.tile as tile
    from concourse import bacc, mybir

    f32 = mybir.dt.float32
    bf16 = mybir.dt.bfloat16
    AF = mybir.ActivationFunctionType
    ALU = mybir.AluOpType

    nc = bacc.Bacc("TRN2", target_bir_lowering=False)

    xT_d = nc.dram_tensor("xT", [C, N], bf16, kind="ExternalInput")
    wqT_d = nc.dram_tensor("wqT", [C, C], bf16, kind="ExternalInput")
    wkT_d = nc.dram_tensor("wkT", [C, C], bf16, kind="ExternalInput")
    wvT_d = nc.dram_tensor("wvT", [C, C], bf16, kind="ExternalInput")
    wpT_d = nc.dram_tensor("wpT", [C, C], bf16, kind="ExternalInput")
    bpT_d = nc.dram_tensor("bpT", [128, CT], f32, kind="ExternalInput")
    biasT8_d = nc.dram_tensor("biasT8", [H, N, N], bf16, kind="ExternalInput")
    outT_d = nc.dram_tensor("outT", [C, N], f32, kind="ExternalOutput")

    # softmax-denominator scratch (row k = (h, nqb) pair)
    s_scr = nc.dram_tensor("s_scr", [H * NQB * F], f32)
    r_scr = nc.dram_tensor("r_scr", [H * NQB, F], bf16)

    with tile.TileContext(nc) as tc:
        with tc.tile_pool(name="persist", bufs=1) as pers:
            xTb = pers.tile([128, CT, N], bf16, tag="xT")
            wqb = pers.tile([128, CT, C], bf16, tag="wq")
            wkb = pers.tile([128, CT, C], bf16, tag="wk")
            wvb = pers.tile([128, CT, C], bf16, tag="wv")
            wpb = pers.tile([128, CT, C], bf16, tag="wp")
            bpb = pers.tile([128, CT], f32, tag="bp")
            # row 64 of this tile collects softmax sums (same start partition
            # as pv[64:65], which the walrus verifier requires)
            s_stage = pers.tile([65, H * NQB * F], f32, tag="s_stage")
            qtb = pers.tile([128, CT, N], bf16, tag="qt")
            ktb = pers.tile([128, CT, N], bf16, tag="kt")
            vb = pers.tile([128, NT, H, D + 1], bf16, tag="v")
            atb = pers.tile([128, CT, N], bf16, tag="at")

            nc.sync.dma_start(xTb, xT_d[:].rearrange("(ci p) n -> p ci n", p=128))
            nc.sync.dma_start(wqb, wqT_d[:].rearrange("(ci p) o -> p ci o", p=128))
            nc.sync.dma_start(wkb, wkT_d[:].rearrange("(ci p) o -> p ci o", p=128))
            nc.sync.dma_start(wvb, wvT_d[:].rearrange("(ci p) o -> p ci o", p=128))
            nc.sync.dma_start(wpb, wpT_d[:].rearrange("(ci p) o -> p ci o", p=128))
            nc.sync.dma_start(bpb, bpT_d[:])

            # ---- projections: V then K then Q --------------------------------
            with tc.tile_pool(name="projps", bufs=3, space="PSUM") as pA:
                # V = x @ Wv^T  -> [n, dv] (n on partitions), scattered into
                # per-head slots of vb with a ones column at dd=64
                for nt in range(NT):
                    for f0, fw, h0 in ((0, 512, 0), (512, 256, 8)):
                        ps = pA.tile([128, F], f32, tag="ps")
                        for ci in range(CT):
                            nc.tensor.matmul(
                                ps[:, :fw],
                                lhsT=xTb[:, ci, nt * 128:(nt + 1) * 128],
                                rhs=wvb[:, ci, f0:f0 + fw],
                                start=(ci == 0),
                                stop=(ci == CT - 1),
                            )
                        nc.vector.tensor_copy(
                            vb[:, nt, h0:h0 + fw // D, 0:D],
                            ps[:, :fw].rearrange("p (h d) -> p h d", d=D),
                        )
                nc.vector.memset(vb[:, :, :, D:D + 1], 1.0)

                # K^T then Q^T: [co, n] (co on partitions)
                for wb, dst in ((wkb, ktb), (wqb, qtb)):
                    for cot in range(CT):
                        for nb in range(NQB):
                            ps = pA.tile([128, F], f32, tag="ps")
                            for ci in range(CT):
                                nc.tensor.matmul(
                                    ps,
                                    lhsT=wb[:, ci, cot * 128:(cot + 1) * 128],
                                    rhs=xTb[:, ci, nb * F:(nb + 1) * F],
                                    start=(ci == 0),
                                    stop=(ci == CT - 1),
                                )
                            nc.vector.tensor_copy(
                                dst[:, cot, nb * F:(nb + 1) * F], ps
                            )

            # ---- attention ---------------------------------------------------
            with tc.tile_pool(name="sps", bufs=3, space="PSUM") as pS, \
                 tc.tile_pool(name="pvps", bufs=2, space="PSUM") as pPV, \
                 tc.tile_pool(name="biasb", bufs=2) as biasp, \
                 tc.tile_pool(name="vstagb", bufs=2) as vstagp, \
                 tc.tile_pool(name="ptb", bufs=3) as ptp:
                for h in range(H):
                    ct, po = h // 2, 64 * (h % 2)
                    for nqb in range(NQB):
                        bt = biasp.tile([128, NT, F], bf16, tag="bt")
                        nc.sync.dma_start(
                            bt,
                            biasT8_d[h].rearrange("(j p) q -> p j q", p=128)[
                                :, :, nqb * F:(nqb + 1) * F
                            ],
                        )
                        pv = pPV.tile([D + 1, F], f32, tag="pv")
                        for j in range(NT):
                            ps = pS.tile([128, F], f32, tag="s")
                            nc.tensor.matmul(
                                ps,
                                lhsT=ktb[po:po + 64, ct, j * 128:(j + 1) * 128],
                                rhs=qtb[po:po + 64, ct, nqb * F:(nqb + 1) * F],
                                start=True,
                                stop=True,
                            )
                            nc.vector.tensor_tensor(ps, ps, bt[:, j, :], ALU.add)
                            pt = ptp.tile([128, F], bf16, tag="pt")
                            nc.scalar.activation(pt, ps, AF.Exp, scale=0.125)
                            nc.tensor.matmul(
                                pv,
                                lhsT=vb[:, j, h, :],
                                rhs=pt,
                                start=(j == 0),
                                stop=(j == NT - 1),
                            )
                        k = 2 * h + nqb
                        dst = atb[po:po + 64, ct, nqb * F:(nqb + 1) * F]
                        if po == 0:
                            nc.vector.tensor_copy(dst, pv[0:D, :])
                        else:
                            # aligned engine copy, then DMA (partition-agnostic)
                            vstag = vstagp.tile([D, F], bf16, tag="vstag")
                            nc.vector.tensor_copy(vstag, pv[0:D, :])
                            nc.sync.dma_start(dst, vstag)
                        nc.scalar.copy(
                            s_stage[D:D + 1, k * F:(k + 1) * F], pv[D:D + 1, :]
                        )

            # ---- batched softmax normalization ------------------------------
            with tc.tile_pool(name="normb", bufs=1) as nrm, \
                 tc.tile_pool(name="rbb", bufs=2) as rbp:
                nc.sync.dma_start(s_scr[:], s_stage[D:D + 1, :])
                sb = nrm.tile([128, 96], f32, tag="sb")
                nc.sync.dma_start(sb, s_scr[:].rearrange("(p f) -> p f", p=128))
                rc32 = nrm.tile([128, 96], f32, tag="rc32")
                nc.vector.reciprocal(rc32, sb)
                rcb = nrm.tile([128, 96], bf16, tag="rcb")
                nc.vector.tensor_copy(rcb, rc32)
                r_flat = r_scr[:].rearrange("k q -> (k q)")
                nc.sync.dma_start(r_flat.rearrange("(p f) -> p f", p=128), rcb)

                for h in range(H):
                    ct, po = h // 2, 64 * (h % 2)
                    for nqb in range(NQB):
                        k = 2 * h + nqb
                        rb = rbp.tile([128, F], bf16, tag="rb")
                        nc.sync.dma_start(
                            rb, r_scr[k:k + 1, :].to_broadcast([128, F])
                        )
                        sl = atb[po:po + 64, ct, nqb * F:(nqb + 1) * F]
                        nc.vector.tensor_tensor(sl, sl, rb[po:po + 64, :], ALU.mult)

            # ---- output projection ------------------------------------------
            with tc.tile_pool(name="ops", bufs=2, space="PSUM") as pC, \
                 tc.tile_pool(name="otb", bufs=3) as otp:
                for cot in range(CT):
                    for nb in range(NQB):
                        ps = pC.tile([128, F], f32, tag="o")
                        for ci in range(CT):
                            nc.tensor.matmul(
                                ps,
                                lhsT=wpb[:, ci, cot * 128:(cot + 1) * 128],
                                rhs=atb[:, ci, nb * F:(nb + 1) * F],
                                start=(ci == 0),
                                stop=(ci == CT - 1),
                            )
                        ot = otp.tile([128, F], f32, tag="ot")
                        nc.scalar.activation(
                            ot, ps, AF.Identity, bias=bpb[:, cot:cot + 1]
                        )
                        nc.sync.dma_start(
                            outT_d[cot * 128:(cot + 1) * 128,
                                   nb * F:(nb + 1) * F],
                            ot,
                        )

    nc.compile()
    return nc


def _get_nc():
    if "nc" not in _cache:
        _cache["nc"] = _build()
    return _cache["nc"]


def prep_in_maps(x, attn_bias, Wq, Wk, Wv, Wp, bp):
    """Host-side sharding + layout prep (transposes/casts only)."""
    wqT = np.ascontiguousarray(Wq.T).astype(BF16)
    wkT = np.ascontiguousarray(Wk.T).astype(BF16)
    wvT = np.ascontiguousarray(Wv.T).astype(BF16)
    wpT = np.ascontiguousarray(Wp.T).astype(BF16)
    bpT = np.ascontiguousarray(
        bp.astype(np.float32).reshape(CT, 128).T
    )
    biasT8 = np.ascontiguousarray(
        (attn_bias[0].astype(np.float32) * 8.0).transpose(0, 2, 1)
    ).astype(BF16)
    in_maps = []
    for b in range(B):
        in_maps.append({
            "xT": np.ascontiguousarray(x[b].T).astype(BF16),
            "wqT": wqT, "wkT": wkT, "wvT": wvT, "wpT": wpT,
            "bpT": bpT, "biasT8": biasT8,
        })
    return in_maps


def run(in_maps, trace=False, **kw):
    from concourse.bass_utils import run_bass_kernel_spmd

    nc = _get_nc()
    return run_bass_kernel_spmd(
        nc, in_maps, core_ids=list(range(B)), trace=trace, **kw
    )


def kernel(x, attn_bias, Wq, Wk, Wv, Wp, bp):
    res = run(prep_in_maps(x, attn_bias, Wq, Wk, Wv, Wp, bp))
    out = np.stack(
        [res.results[b]["outT"].T for b in range(B)]
    ).astype(np.float32)
    return out


# revision 16
# speedup vs baseline: 1.1816x; 1.1816x over previous
"""Multi-head attention (B=8, N=1024, C=768, H=12, D=64) on 8 TRN2 NeuronCores.

Strategy: pure data-parallel over batch (B == n_cores == 8), no collectives.
Each core computes full 12-head attention for one batch element, in a fully
transposed layout (channels on SBUF partitions) so no on-device transposes are
needed:

  per core:  xT=[C,N] -> QT,KT=[C,N], V=[N,C] (+ ones col)
             per (head, nk-tile): S^T = K_h Q_h^T  into PSUM [128 nk, 1024 nq]
             S^T += 8*bias^T (DVE);  P^T = exp(0.125*S^T) (ACT -> bf16)
             PV:  [V_h | 1]^T @ P^T -> rows 0:64 = out_h^T (unnorm), row 64 = sum
             batched reciprocal of all softmax sums, one broadcast DMA,
             normalize, out^T = Wp @ attnT + bp -> DMA out, host transposes.

K/Q projection tiles are interleaved into the attention emission so the
TensorEngine stays dense (HAM stays un-throttled). Matmuls in bf16, f32 PSUM.
"""

import os
import sys
import numpy as np

for _p in ("/opt/trn_rl_repo", "/root/.axon_site/_ro/trn_rl_repo"):
    if os.path.isdir(_p) and _p not in sys.path:
        sys.path.append(_p)

import ml_dtypes

BF16 = ml_dtypes.bfloat16

B, N, C = 8, 1024, 768
H, D = 12, 64
CT = C // 128        # 6 channel tiles
NT = N // 128        # 8 key tiles
F = 512

_cache = {}


def _build():
    import concourse.bass as bass
    import concourse.tile as tile
    from concourse import bacc, mybir

    f32 = mybir.dt.float32
    bf16 = mybir.dt.bfloat16
    AF = mybir.ActivationFunctionType
    ALU = mybir.AluOpType

    nc = bacc.Bacc("TRN2", target_bir_lowering=False)

    xT_d = nc.dram_tensor("xT", [C, N], bf16, kind="ExternalInput")
    wqT_d = nc.dram_tensor("wqT", [C, C], bf16, kind="ExternalInput")
    wkT_d = nc.dram_tensor("wkT", [C, C], bf16, kind="ExternalInput")
    wvT_d = nc.dram_tensor("wvT", [C, C], bf16, kind="ExternalInput")
    wpT_d = nc.dram_tensor("wpT", [C, C], bf16, kind="ExternalInput")
    bpT_d = nc.dram_tensor("bpT", [128, CT], f32, kind="ExternalInput")
    biasT8_d = nc.dram_tensor("biasT8", [H, N, N], bf16, kind="ExternalInput")
    outT_d = nc.dram_tensor("outT", [C, N], f32, kind="ExternalOutput")

    s_scr = nc.dram_tensor("s_scr", [H * N], f32)
    r_scr = nc.dram_tensor("r_scr", [1, H * N], bf16)

    with tile.TileContext(nc) as tc:
        with tc.tile_pool(name="persist", bufs=1) as pers:
            xTb = pers.tile([128, CT, N], bf16, tag="xT")
            wqb = pers.tile([128, CT, C], bf16, tag="wq")
            wkb = pers.tile([128, CT, C], bf16, tag="wk")
            wvb = pers.tile([128, CT, C], bf16, tag="wv")
            wpb = pers.tile([128, CT, C], bf16, tag="wp")
            bpb = pers.tile([128, CT], f32, tag="bp")
            # row 64 collects softmax sums (same start partition as pv[64:65])
            s_stage = pers.tile([65, H * N], f32, tag="s_stage")
            qtb = pers.tile([128, CT, N], bf16, tag="qt")
            ktb = pers.tile([128, CT, N], bf16, tag="kt")
            vb = pers.tile([128, NT, H, D + 1], bf16, tag="v")
            atb = pers.tile([128, CT, N], bf16, tag="at")
            rba = pers.tile([128, H * N], bf16, tag="rba")

            # inputs: per-channel-tile loads so compute can start early
            for ci in range(CT):
                nc.sync.dma_start(
                    xTb[:, ci, :], xT_d[ci * 128:(ci + 1) * 128, :])
                nc.sync.dma_start(
                    wvb[:, ci, :], wvT_d[ci * 128:(ci + 1) * 128, :])
                nc.sync.dma_start(
                    wkb[:, ci, :], wkT_d[ci * 128:(ci + 1) * 128, :])
                nc.sync.dma_start(
                    wqb[:, ci, :], wqT_d[ci * 128:(ci + 1) * 128, :])
            nc.scalar.dma_start(bpb, bpT_d[:])
            for ci in range(CT):
                nc.scalar.dma_start(
                    wpb[:, ci, :], wpT_d[ci * 128:(ci + 1) * 128, :])

            nc.vector.memset(vb[:, :, :, D:D + 1], 1.0)

            with tc.tile_pool(name="kqps", bufs=2, space="PSUM") as pA, \
                 tc.tile_pool(name="sps", bufs=2, space="PSUM") as pS, \
                 tc.tile_pool(name="pvps", bufs=1, space="PSUM") as pPV, \
                 tc.tile_pool(name="biasb", bufs=2) as biasp, \
                 tc.tile_pool(name="vstagb", bufs=2) as vstagp, \
                 tc.tile_pool(name="ptb", bufs=2) as ptp:

                def v_proj(block):  # block 0: heads 0-7, block 1: heads 8-11
                    f0, fw, h0 = (0, 512, 0) if block == 0 else (512, 256, 8)
                    for nt in range(NT):
                        ps = pA.tile([128, F], f32, tag="ps")
                        for ci in range(CT):
                            nc.tensor.matmul(
                                ps[:, :fw],
                                lhsT=xTb[:, ci, nt * 128:(nt + 1) * 128],
                                rhs=wvb[:, ci, f0:f0 + fw],
                                start=(ci == 0),
                                stop=(ci == CT - 1),
                            )
                        nc.vector.tensor_copy(
                            vb[:, nt, h0:h0 + fw // D, 0:D],
                            ps[:, :fw].rearrange("p (h d) -> p h d", d=D),
                        )

                def kq_proj(cot):  # K^T and Q^T channel tile cot
                    for wb, dst in ((wkb, ktb), (wqb, qtb)):
                        for nb in range(2):
                            ps = pA.tile([128, F], f32, tag="ps")
                            for ci in range(CT):
                                nc.tensor.matmul(
                                    ps,
                                    lhsT=wb[:, ci, cot * 128:(cot + 1) * 128],
                                    rhs=xTb[:, ci, nb * F:(nb + 1) * F],
                                    start=(ci == 0),
                                    stop=(ci == CT - 1),
                                )
                            nc.vector.tensor_copy(
                                dst[:, cot, nb * F:(nb + 1) * F], ps)

                def attn(h):
                    ct, po = h // 2, 64 * (h % 2)
                    bt = biasp.tile([128, NT, N], bf16, tag="bt")
                    nc.sync.dma_start(
                        bt, biasT8_d[h].rearrange("(j p) q -> p j q", p=128))
                    pv0 = pPV.tile([D + 1, F], f32, tag="pv0")
                    pv1 = pPV.tile([D + 1, F], f32, tag="pv1")
                    for j in range(NT):
                        ps = pS.tile([128, N], f32, tag="s")
                        for nb in range(2):
                            nc.tensor.matmul(
                                ps[:, nb * F:(nb + 1) * F],
                                lhsT=ktb[po:po + 64, ct, j * 128:(j + 1) * 128],
                                rhs=qtb[po:po + 64, ct, nb * F:(nb + 1) * F],
                                start=True,
                                stop=True,
                            )
                        nc.vector.tensor_tensor(ps, ps, bt[:, j, :], ALU.add)
                        pt = ptp.tile([128, N], bf16, tag="pt")
                        nc.scalar.activation(pt, ps, AF.Exp, scale=0.125)
                        for nb, pv in ((0, pv0), (1, pv1)):
                            nc.tensor.matmul(
                                pv,
                                lhsT=vb[:, j, h, :],
                                rhs=pt[:, nb * F:(nb + 1) * F],
                                start=(j == 0),
                                stop=(j == NT - 1),
                            )
                    for nb, pv in ((0, pv0), (1, pv1)):
                        dst = atb[po:po + 64, ct, nb * F:(nb + 1) * F]
                        if po == 0:
                            nc.vector.tensor_copy(dst, pv[0:D, :])
                        else:
                            vstag = vstagp.tile([D, F], bf16, tag="vstag")
                            nc.vector.tensor_copy(vstag, pv[0:D, :])
                            nc.gpsimd.dma_start(dst, vstag)
                        nc.scalar.copy(
                            s_stage[D:D + 1, h * N + nb * F:
                                    h * N + (nb + 1) * F],
                            pv[D:D + 1, :],
                        )

                # emission order: keep PE dense; kq_proj(ct) before attn(2ct)
                v_proj(0)
                kq_proj(0)
                attn(0)
                kq_proj(1)
                attn(1)
                kq_proj(2)
                attn(2)
                kq_proj(3)
                attn(3)
                v_proj(1)
                kq_proj(4)
                attn(4)
                kq_proj(5)
                for h in range(5, H):
                    attn(h)

            # ---- batched softmax normalization ------------------------------
            with tc.tile_pool(name="normb", bufs=1) as nrm:
                nc.scalar.dma_start(s_scr[:], s_stage[D:D + 1, :])
                sb = nrm.tile([128, H * N // 128], f32, tag="sb")
                nc.scalar.dma_start(
                    sb, s_scr[:].rearrange("(p f) -> p f", p=128))
                rc32 = nrm.tile([128, H * N // 128], f32, tag="rc32")
                nc.vector.reciprocal(rc32, sb)
                rcb = nrm.tile([128, H * N // 128], bf16, tag="rcb")
                nc.vector.tensor_copy(rcb, rc32)
                nc.scalar.dma_start(
                    r_scr[0, :].rearrange("(p f) -> p f", p=128), rcb)
                # one broadcast DMA for all heads' reciprocals
                nc.sync.dma_start(rba, r_scr[:].to_broadcast([128, H * N]))

            # ---- normalize + output projection, interleaved by ct -----------
            with tc.tile_pool(name="ops", bufs=3, space="PSUM") as pC, \
                 tc.tile_pool(name="otb", bufs=3) as otp:
                for cot in range(CT):
                    for h in (2 * cot, 2 * cot + 1):
                        po = 64 * (h % 2)
                        sl = atb[po:po + 64, cot, :]
                        nc.vector.tensor_tensor(
                            sl, sl, rba[po:po + 64, h * N:(h + 1) * N],
                            ALU.mult)
                for cot in range(CT):
                    for nb in range(2):
                        ps = pC.tile([128, F], f32, tag="o")
                        for ci in range(CT):
                            nc.tensor.matmul(
                                ps,
                                lhsT=wpb[:, ci, cot * 128:(cot + 1) * 128],
                                rhs=atb[:, ci, nb * F:(nb + 1) * F],
                                start=(ci == 0),
                                stop=(ci == CT - 1),
                            )
                        ot = otp.tile([128, F], f32, tag="ot")
                        nc.scalar.activation(
                            ot, ps, AF.Identity, bias=bpb[:, cot:cot + 1])
                        nc.scalar.dma_start(
                            outT_d[cot * 128:(cot + 1) * 128,
                                   nb * F:(nb + 1) * F],
                            ot,
                        )

    nc.compile()
    return nc


def _get_nc():
    if "nc" not in _cache:
        _cache["nc"] = _build()
    return _cache["nc"]


def prep_in_maps(x, attn_bias, Wq, Wk, Wv, Wp, bp):
    """Host-side sharding + layout prep (transposes/casts only)."""
    wqT = np.ascontiguousarray(Wq.T).astype(BF16)
    wkT = np.ascontiguousarray(Wk.T).astype(BF16)
    wvT = np.ascontiguousarray(Wv.T).astype(BF16)
    wpT = np.ascontiguousarray(Wp.T).astype(BF16)
    bpT = np.ascontiguousarray(bp.astype(np.float32).reshape(CT, 128).T)
    biasT8 = np.ascontiguousarray(
        (attn_bias[0].astype(np.float32) * 8.0).transpose(0, 2, 1)
    ).astype(BF16)
    in_maps = []
    for b in range(B):
        in_maps.append({
            "xT": np.ascontiguousarray(x[b].T).astype(BF16),
            "wqT": wqT, "wkT": wkT, "wvT": wvT, "wpT": wpT,
            "bpT": bpT, "biasT8": biasT8,
        })
    return in_maps


def run(in_maps, trace=False, **kw):
    from concourse.bass_utils import run_bass_kernel_spmd

    nc = _get_nc()
    return run_bass_kernel_spmd(
        nc, in_maps, core_ids=list(range(B)), trace=trace, **kw
    )


def kernel(x, attn_bias, Wq, Wk, Wv, Wp, bp):
    res = run(prep_in_maps(x, attn_bias, Wq, Wk, Wv, Wp, bp))
    out = np.stack(
        [res.results[b]["outT"].T for b in range(B)]
    ).astype(np.float32)
    return out
